# revision 1
# baseline (speedup 1.0000x reference)
"""CycleMatcher (mutual-nearest-neighbor descriptor matching) on 8 trn2 cores.

Problem: B=4 pairs of L2-normalized descriptor sets d0,d1 [8192, 64].
dist = sqrt2*sqrt(clip(1 - d0@d1.T, 1e-6)) ; row/col argmins; mutual-NN
masking; scatter. dist is monotone-decreasing in sim = d0@d1.T, so argmin
dist == argmax sim (with care for fp32 sqrt rounding ties, resolved on host).

Sharding: 8 cores = 4 batches x 2 orientations. Core (b, 0) computes
S = d0[b] @ d1[b].T row-argmax (n_amin side); core (b, 1) computes
S.T = d1[b] @ d0[b].T row-argmax (m_amin side). Identical device program,
inputs swapped.

Device program per core: for each 128-row strip (64 strips), fp32 matmuls
[64,128]^T @ [64,512] fill PSUM in [128, 2048] groups (4 banks, double
buffered); ScalarE drains each group to SBUF; DVE `max` (top-8 values) +
`max_index` (their indices) reduce each SBUF group. Exports per row
4 groups x top-8 (value, local index) candidates. Host merges candidates,
resolves sqrt-rounding ties exactly in reference fp32 semantics
(fp64-refining near-ties), then does the cheap mutual-NN match + scatter
in numpy. Measured device time ~1.17 ms (DVE-bound: 2 passes over 67M
fp32 elements at 1 elem/cycle/lane, 0.96 GHz).
"""

import os
import sys

# Prefer whatever copy PYTHONPATH already provides (the axon sitecustomize
# puts /root/.axon_site/_ro/trn_rl_repo there); append fallbacks so kernel.py
# also works standalone without creating dual module identities.
for _p in ("/root/.axon_site/_ro/trn_rl_repo", "/opt/trn_rl_repo"):
    if _p not in sys.path:
        sys.path.append(_p)

import numpy as np

import concourse.bass as bass
import concourse.mybir as mybir
import concourse.tile as tile
from concourse import bacc
from concourse.bass_utils import run_bass_kernel_spmd

B = 4
M = 8192
N = 8192
D = 64

PART = 128          # rows per strip (psum partitions)
NSTRIP = M // PART  # 64
MMN = 512           # matmul moving free dim (one psum bank, fp32)
GRP = int(os.environ.get("KERNEL_GRP", "2048"))  # psum group / DVE op width
NG = N // GRP       # 4 groups per strip
TOPK = 8            # DVE max/max_index width

# Variant is needed before CAND can be fixed (strip variant exports one
# top-8 per row, group variants export one per [128, GRP] group).
_VARIANT_ENV = os.environ.get("KERNEL_VARIANT", "sbuf")
# candidate groups per row by variant: (n_groups, group_width)
_GROUPS = {"strip": (1, N), "sbuf4k": (2, 2 * GRP)}.get(_VARIANT_ENV, (NG, GRP))
CAND = _GROUPS[0] * TOPK

SQRT_2 = np.float32(1.414213)

# Ablation for differential timing only: 0 = full, 1 = no max_index,
# 2 = no max/max_index (matmuls only). Never set for real runs.
_ABLATE = int(os.environ.get("KERNEL_ABLATE", "0"))
# Variants (KERNEL_VARIANT):
#   psum:  DVE max/max_index read PSUM groups directly (1.54 ms measured)
#   sbuf:  ScalarE drains each PSUM group to SBUF; DVE reduces [128,2048]
#          SBUF groups (1.17 ms — PSUM-sourced DVE ops pay extra access
#          overhead and contend with PE writes; ScalarE is otherwise idle)
#   strip: like sbuf but DVE reduces whole [128,8192] strips in one
#          max + one max_index (measured 3.6 ms - large DVE ops pay
#          duration-proportional DRAIN; do not use)
_VARIANT = _VARIANT_ENV

_prog_cache = {}


def _build_program():
    # KERNEL_REPEATS > 1 repeats the whole compute loop (unrolled);
    # KERNEL_LOOP > 1 wraps it in an on-device For_i (constant program size).
    # Both are only for differential wall-clock timing: axon dispatch
    # overhead dominates a single run, the slope over repeats isolates
    # device time.
    repeats = int(os.environ.get("KERNEL_REPEATS", "1"))
    loops = int(os.environ.get("KERNEL_LOOP", "1"))
    nc = bacc.Bacc("TRN2", target_bir_lowering=False, debug=False)
    f32 = mybir.dt.float32
    u32 = mybir.dt.uint32

    at_d = nc.dram_tensor("at", [D, M], f32, kind="ExternalInput")
    bt_d = nc.dram_tensor("bt", [D, N], f32, kind="ExternalInput")
    vals_d = nc.dram_tensor("vals", [PART, NSTRIP * CAND], f32, kind="ExternalOutput")
    idxs_d = nc.dram_tensor("idxs", [PART, NSTRIP * CAND], u32, kind="ExternalOutput")

    with tile.TileContext(nc) as tc:
        with (
            tc.tile_pool(name="inp", bufs=1) as inp,
            tc.tile_pool(name="outp", bufs=1) as outp,
            tc.tile_pool(name="ps", bufs=2, space="PSUM") as ps,
            tc.tile_pool(
                name="stage",
                bufs={"strip": 2, "sbuf2": 8, "sbuf4k": 3}.get(_VARIANT, 4),
            ) as stage,
        ):
            at = inp.tile([D, M], f32)
            bt = inp.tile([D, N], f32)
            # two different HWDGE queues so the loads overlap
            nc.sync.dma_start(at[:], at_d.ap())
            nc.scalar.dma_start(bt[:], bt_d.ap())

            vals = outp.tile([PART, NSTRIP * CAND], f32)
            idxs = outp.tile([PART, NSTRIP * CAND], u32)

            def body():
                for m in [mm % NSTRIP for mm in range(NSTRIP * repeats)]:
                    lhsT = at[:, m * PART:(m + 1) * PART]  # [64, 128] stationary
                    strip = None
                    if _VARIANT == "strip":
                        strip = stage.tile([PART, N], f32, tag="strip")
                    sts = []
                    for g in range(NG):
                        pt = ps.tile([PART, GRP], f32)
                        for j in range(GRP // MMN):
                            n0 = g * GRP + j * MMN
                            nc.tensor.matmul(
                                pt[:, j * MMN:(j + 1) * MMN],
                                lhsT,
                                bt[:, n0:n0 + MMN],
                                start=True,
                                stop=True,
                            )
                        if _VARIANT == "strip":
                            nc.scalar.copy(strip[:, g * GRP:(g + 1) * GRP], pt[:])
                            continue
                        if _VARIANT == "sbuf4k":
                            # two PSUM groups share one [128, 4096] stage
                            # tile; DVE reduces it in one max + max_index
                            if g % 2 == 0:
                                st4 = stage.tile([PART, 2 * GRP], f32, tag="st4")
                            nc.scalar.copy(
                                st4[:, (g % 2) * GRP:(g % 2 + 1) * GRP], pt[:]
                            )
                            if g % 2 == 1:
                                gg = g // 2
                                c0 = m * CAND + gg * TOPK
                                vs = vals[:, c0:c0 + TOPK]
                                nc.vector.max(out=vs, in_=st4[:])
                                nc.vector.max_index(
                                    out=idxs[:, c0:c0 + TOPK],
                                    in_max=vs,
                                    in_values=st4[:],
                                )
                            continue
                        if _VARIANT == "sbuf2":
                            # drain now; issue all max ops, then all
                            # max_index ops after the group loop so
                            # independent work sits between dependent pairs
                            st = stage.tile([PART, GRP], f32, tag="st2")
                            nc.scalar.copy(st[:], pt[:])
                            sts.append(st)
                            continue
                        c0 = m * CAND + g * TOPK
                        vs = vals[:, c0:c0 + TOPK]
                        src = pt
                        if _VARIANT == "sbuf":
                            st = stage.tile([PART, GRP], f32)
                            nc.scalar.copy(st[:], pt[:])
                            src = st
                        if _ABLATE >= 2:
                            # keep the matmuls live with a minimal psum read
                            nc.vector.tensor_copy(vals[:, c0:c0 + 1], pt[:, 0:1])
                        if _ABLATE < 2:
                            nc.vector.max(out=vs, in_=src[:])
                        if _ABLATE < 1:
                            nc.vector.max_index(
                                out=idxs[:, c0:c0 + TOPK], in_max=vs, in_values=src[:]
                            )
                    if _VARIANT == "strip":
                        c0 = m * TOPK
                        vs = vals[:, c0:c0 + TOPK]
                        nc.vector.max(out=vs, in_=strip[:])
                        nc.vector.max_index(
                            out=idxs[:, c0:c0 + TOPK], in_max=vs, in_values=strip[:]
                        )
                    if _VARIANT == "sbuf2":
                        for g in range(NG):
                            c0 = m * CAND + g * TOPK
                            nc.vector.max(out=vals[:, c0:c0 + TOPK], in_=sts[g][:])
                        for g in range(NG):
                            c0 = m * CAND + g * TOPK
                            nc.vector.max_index(
                                out=idxs[:, c0:c0 + TOPK],
                                in_max=vals[:, c0:c0 + TOPK],
                                in_values=sts[g][:],
                            )

            if loops > 1:
                with tc.For_i(0, loops, 1):
                    body()
            else:
                body()

            nc.sync.dma_start(vals_d.ap(), vals[:])
            nc.scalar.dma_start(idxs_d.ap(), idxs[:])

    nc.compile()
    return nc


def _get_program():
    if "nc" not in _prog_cache:
        _prog_cache["nc"] = _build_program()
    return _prog_cache["nc"]


def _dist32(sim):
    """Reference fp32 distance pipeline: sqrt2 * sqrt(clip(1 - sim, 1e-6))."""
    sim = np.asarray(sim, dtype=np.float32)
    t = np.clip(np.float32(1.0) - sim, np.float32(1e-6), None).astype(np.float32)
    return (SQRT_2 * np.sqrt(t)).astype(np.float32)


def _select_winners(vals, idxs, a64, b64):
    """Pick per-row argmin-of-dist winners from top-8-per-group candidates.

    vals, idxs: [PART, NSTRIP*CAND] device outputs for one core.
    a64, b64: fp64 copies of the descriptor sets (rows of S are a64 @ b64.T),
      used to refine rows where fp32 candidate sims are too close to call.
    Returns (win_idx int64 [M], win_sim float32 [M]).
    """
    # [p, m, g, k] -> row r = m*PART + p; group offsets per variant
    ng, gw = _GROUPS
    V = vals.reshape(PART, NSTRIP, ng, TOPK).transpose(1, 0, 2, 3).reshape(M, CAND)
    I = idxs.reshape(PART, NSTRIP, ng, TOPK).transpose(1, 0, 2, 3).astype(np.int64)
    I += np.arange(ng, dtype=np.int64)[None, None, :, None] * gw
    I = I.reshape(M, CAND)

    rows = np.arange(M)

    def pick(Vc, Ic):
        dist = _dist32(Vc)
        dmin = dist.min(axis=1, keepdims=True)
        tie = dist == dmin
        gi = np.where(tie, Ic, np.int64(1) << 40)
        widx = gi.min(axis=1)
        wpos = np.argmax(tie & (gi == widx[:, None]), axis=1)
        return widx, Vc[rows[: len(Vc)], wpos]

    win_idx, win_sim = pick(V, I)

    # Rows where several candidates sit within fp32-rounding distance of the
    # max: recompute their candidate sims in fp64 and redo the fp32 pipeline,
    # mirroring what the reference's own fp32 matmul would produce.
    vmax = V.max(axis=1, keepdims=True)
    near = (vmax - V) < np.float32(3e-5)
    amb = np.flatnonzero(near.sum(axis=1) > 1)
    if os.environ.get("KERNEL_DEBUG"):
        print(f"[kernel] rows fp64-refined: {amb.size}/{len(V)}")
    if amb.size:
        Ic = np.clip(I[amb], 0, b64.shape[0] - 1)
        sims64 = np.einsum(
            "rd,rcd->rc", a64[amb], b64[Ic], optimize=True
        )
        V2 = sims64.astype(np.float32)
        w2, s2 = pick(V2, I[amb])
        win_idx[amb] = w2
        win_sim[amb] = s2

    return win_idx, win_sim


def _match_batch_host(row_res, col_res, d0b, d1b):
    """Reproduce reference _match_batch from the two cores' candidate lists."""
    d0_64 = d0b.astype(np.float64)
    d1_64 = d1b.astype(np.float64)
    n_amin, sim_row = _select_winners(row_res["vals"], row_res["idxs"], d0_64, d1_64)
    m_amin, _ = _select_winners(col_res["vals"], col_res["idxs"], d1_64, d0_64)

    rng_m = np.arange(M, dtype=np.int64)
    mask = m_amin[n_amin] == rng_m

    dist_w = _dist32(sim_row)
    score = (np.float32(1.0) / (np.float32(1.0) + dist_w)).astype(np.float32)

    m0 = np.where(mask, n_amin, -1).astype(np.int32)
    ms0 = np.where(mask, score, np.float32(0.0)).astype(np.float32)

    m1 = np.full(N, -1, dtype=np.int32)
    ms1 = np.zeros(N, dtype=np.float32)
    sel = np.flatnonzero(mask)
    m1[n_amin[sel]] = sel.astype(np.int32)
    ms1[n_amin[sel]] = score[sel]
    return m0, ms0, m1, ms1


def _build_in_maps(desc0, desc1):
    d0T = np.ascontiguousarray(desc0.transpose(0, 2, 1))  # [B, 64, M]
    d1T = np.ascontiguousarray(desc1.transpose(0, 2, 1))  # [B, 64, N]
    in_maps = []
    for b in range(B):
        in_maps.append({"at": d0T[b], "bt": d1T[b]})  # row side (o=0)
        in_maps.append({"at": d1T[b], "bt": d0T[b]})  # col side (o=1)
    return in_maps


def run_device(in_maps, trace=False):
    nc = _get_program()
    return run_bass_kernel_spmd(nc, in_maps, core_ids=list(range(8)), trace=trace)


def kernel(kpts0, desc0, kpts1, desc1):
    desc0 = np.asarray(desc0, dtype=np.float32)
    desc1 = np.asarray(desc1, dtype=np.float32)
    assert desc0.shape == (B, M, D) and desc1.shape == (B, N, D)

    in_maps = _build_in_maps(desc0, desc1)
    trace = bool(int(os.environ.get("KERNEL_PROFILE", "0")))
    res = run_device(in_maps, trace=trace)
    kernel.last_results = res
    kernel.last_exec_time_ns = res.exec_time_ns

    m0 = np.empty((B, M), np.int32)
    ms0 = np.empty((B, M), np.float32)
    m1 = np.empty((B, N), np.int32)
    ms1 = np.empty((B, N), np.float32)
    for b in range(B):
        m0[b], ms0[b], m1[b], ms1[b] = _match_batch_host(
            res.results[2 * b], res.results[2 * b + 1], desc0[b], desc1[b]
        )
    return m0, ms0, m1, ms1



# revision 3
# speedup vs baseline: 7.0931x; 7.0931x over previous
"""CycleMatcher (mutual-nearest-neighbor descriptor matching) on trn2 via axon.

Problem: B=4 pairs of L2-normalized descriptor sets d0,d1 [8192, 64].
dist = sqrt2*sqrt(clip(1 - d0@d1.T, 1e-6)); row/col argmins; mutual-NN
masking; scatter. dist is monotone-decreasing in sim = d0@d1.T, so
argmin dist == argmax sim (fp32 sqrt-rounding ties replicated on host).

The axon tunnel moves ~36-40 MB/s aggregate (serialized — per-device
streams do NOT parallelize) with ~60ms fixed dispatch latency, while the
device itself needs only a few ms. So the whole design minimizes tunnel
bytes:

  * 4 cores, one batch each, BOTH matrix orientations per core, so each
    batch's descriptors cross the tunnel once: at=d0[b].T, bt=d1[b].T as
    [64, 8192] fp16 (1MB each) -> 8MB total upload (vs 32MB fp32
    data-parallel with a separate col-argmin core).
  * Device finishes the argmax candidate search: per 128-row strip, fp16
    matmuls fill a [128, 8192] fp32 sim strip (PSUM group drained by
    ScalarE), DVE max + max_index export the top-8 column indices per
    row as u16. Output = indices only: [128, 2*64*8] u16 = 256KB/core,
    1MB total download (vs 16MB of (val,idx) candidates).
  * One dispatch through a CACHED jitted shard_map callable (the stock
    run_bass_kernel_spmd re-traces jax.jit every call), donating the
    previous call's output buffers so no zero-buffers are uploaded.

Host post-processing re-evaluates all 8 candidates per row in fp64 from
the original fp32 descriptors and replays the reference's exact fp32
dist pipeline (clip/sqrt rounding, first-index argmin ties), so device
fp16 matmul noise only matters if the TRUE argmax drops out of the
device top-8 — needs 8 columns within ~1e-3 of the row max, which for
these descriptor statistics has probability ~0 (verified exact vs the
reference on the fixed harness inputs).
"""

import os
import sys

# Prefer whatever copy PYTHONPATH already provides (the axon sitecustomize
# puts /root/.axon_site/_ro/trn_rl_repo there); append fallbacks so kernel.py
# also works standalone without creating dual module identities.
for _p in ("/root/.axon_site/_ro/trn_rl_repo", "/opt/trn_rl_repo"):
    if _p not in sys.path:
        sys.path.append(_p)

import numpy as np

import concourse.bass as bass
import concourse.mybir as mybir
import concourse.tile as tile
from concourse import bacc

B = 4
M = 8192
N = 8192
D = 64

PART = 128          # rows per strip (psum partitions)
NSTRIP = M // PART  # 64
MMN = 512           # matmul moving free dim (one psum bank, fp32)
GRP = 2048          # psum group width (4 banks)
NG = N // GRP       # 4 groups per strip
TOPK = 8            # DVE max/max_index width
N_CORES = 4

SQRT_2 = np.float32(1.414213)

_cache = {}


def _build_program():
    nc = bacc.Bacc("TRN2", target_bir_lowering=False, debug=False)
    f16 = mybir.dt.float16
    f32 = mybir.dt.float32
    u16 = mybir.dt.uint16

    at_d = nc.dram_tensor("at", [D, M], f16, kind="ExternalInput")
    bt_d = nc.dram_tensor("bt", [D, N], f16, kind="ExternalInput")
    # top-8 column indices per row; cols [phase*NSTRIP*TOPK + m*TOPK : +TOPK]
    idx_d = nc.dram_tensor("idx", [PART, 2 * NSTRIP * TOPK], u16,
                           kind="ExternalOutput")

    with tile.TileContext(nc) as tc:
        with (
            tc.tile_pool(name="inp", bufs=1) as inp,
            tc.tile_pool(name="outp", bufs=1) as outp,
            tc.tile_pool(name="ps", bufs=2, space="PSUM") as ps,
            tc.tile_pool(name="strip", bufs=2) as stage,
            tc.tile_pool(name="vals", bufs=4) as vpool,
        ):
            at = inp.tile([D, M], f16)
            bt = inp.tile([D, N], f16)
            # two different HWDGE queues so the loads overlap
            nc.sync.dma_start(at[:], at_d.ap())
            nc.scalar.dma_start(bt[:], bt_d.ap())

            idx = outp.tile([PART, 2 * NSTRIP * TOPK], u16)

            for phase, (lhs_src, rhs_src) in enumerate(((at, bt), (bt, at))):
                for m in range(NSTRIP):
                    lhsT = lhs_src[:, m * PART:(m + 1) * PART]  # [64, 128]
                    strip = stage.tile([PART, N], f32, tag="strip")
                    for g in range(NG):
                        pt = ps.tile([PART, GRP], f32)
                        for j in range(GRP // MMN):
                            n0 = g * GRP + j * MMN
                            nc.tensor.matmul(
                                pt[:, j * MMN:(j + 1) * MMN],
                                lhsT,
                                rhs_src[:, n0:n0 + MMN],
                                start=True,
                                stop=True,
                            )
                        nc.scalar.copy(strip[:, g * GRP:(g + 1) * GRP], pt[:])
                    c0 = (phase * NSTRIP + m) * TOPK
                    vs = vpool.tile([PART, TOPK], f32, tag="vs")
                    nc.vector.max(out=vs[:], in_=strip[:])
                    nc.vector.max_index(
                        out=idx[:, c0:c0 + TOPK], in_max=vs[:], in_values=strip[:]
                    )

            nc.sync.dma_start(idx_d.ap(), idx[:])

    nc.compile()
    return nc


def _get_runner():
    """Build (once) and return a cached jitted SPMD callable.

    Returns (fn, out_shape): fn(at_global, bt_global, donate_buf) -> idx_global
    where *_global stack the 4 cores on axis 0 and donate_buf is any device or
    host array of the output's global shape/dtype (contents ignored — the
    kernel fully overwrites it; pass the previous call's output to avoid
    uploading zeros).
    """
    if "runner" in _cache:
        return _cache["runner"]

    import jax
    from jax.sharding import Mesh, PartitionSpec
    from jax.experimental.shard_map import shard_map  # matches bass2jax
    from concourse.bass2jax import (
        _bass_exec_p,
        install_neuronx_cc_hook,
        partition_id_tensor,
    )

    nc = _build_program()
    install_neuronx_cc_hook()

    partition_name = nc.partition_id_tensor.name if nc.partition_id_tensor else None
    in_names, out_names, out_avals = [], [], []
    for alloc in nc.m.functions[0].allocations:
        if not isinstance(alloc, mybir.MemoryLocationSet):
            continue
        name = alloc.memorylocations[0].name
        if alloc.kind == "ExternalInput":
            if name != partition_name:
                in_names.append(name)
        elif alloc.kind == "ExternalOutput":
            out_names.append(name)
            out_avals.append(
                jax.core.ShapedArray(tuple(alloc.tensor_shape),
                                     mybir.dt.np(alloc.dtype))
            )
    assert in_names == ["at", "bt"] and out_names == ["idx"], (in_names, out_names)
    n_params = len(in_names)
    all_in_names = in_names + out_names
    if partition_name is not None:
        all_in_names = all_in_names + [partition_name]

    def _body(*args):
        operands = list(args)
        if partition_name is not None:
            operands.append(partition_id_tensor())
        outs = _bass_exec_p.bind(
            *operands,
            out_avals=tuple(out_avals),
            in_names=tuple(all_in_names),
            out_names=tuple(out_names),
            lowering_input_output_aliases=(),
            sim_require_finite=True,
            sim_require_nnan=True,
            nc=nc,
        )
        return tuple(outs)

    devices = jax.devices()[:N_CORES]
    mesh = Mesh(np.asarray(devices), ("core",))
    n_outs = len(out_names)
    sharded = jax.jit(
        shard_map(
            _body,
            mesh=mesh,
            in_specs=(PartitionSpec("core"),) * (n_params + n_outs),
            out_specs=(PartitionSpec("core"),) * n_outs,
            check_rep=False,
        ),
        donate_argnums=(n_params,),
        keep_unused=True,
    )

    def fn(at_g, bt_g, donate):
        (out,) = sharded(at_g, bt_g, donate)
        return out

    out_shape = (N_CORES * PART, 2 * NSTRIP * TOPK)
    _cache["runner"] = (fn, out_shape)
    return _cache["runner"]


def stage_inputs(desc0, desc1):
    """Host-side staging: per-batch transposed fp16 descriptor planes.

    Returns (at_global, bt_global): [N_CORES*D, M] fp16, core-major.
    """
    at_g = np.empty((N_CORES * D, M), np.float16)
    bt_g = np.empty((N_CORES * D, N), np.float16)
    for b in range(B):
        at_g[b * D:(b + 1) * D] = desc0[b].astype(np.float16).T
        bt_g[b * D:(b + 1) * D] = desc1[b].astype(np.float16).T
    return at_g, bt_g


def run_device(at_g, bt_g):
    """Upload staged inputs, run the 4-core program, fetch index candidates.

    Returns idx_global [N_CORES*PART, 2*NSTRIP*TOPK] u16 as host numpy.
    """
    fn, out_shape = _get_runner()
    donate = _cache.get("donate")
    if donate is None:
        donate = np.zeros(out_shape, np.uint16)
    out = fn(at_g, bt_g, donate)
    res = np.asarray(out)
    _cache["donate"] = out  # recycle device buffer for the next call
    return res


def _pick_winners(cand_idx, a64, b64):
    """argmin-of-dist winner per row from top-8 candidate indices.

    cand_idx: [M, TOPK] int64 candidate columns for each row.
    a64, b64: fp64 descriptor sets (sim[r, c] = a64[r] @ b64[c]).
    Replicates the reference fp32 pipeline: sims -> fp32 ->
    dist = sqrt2*sqrt(clip(1-sim, 1e-6)) -> argmin, first-index ties.
    Returns (win_idx int64 [M], win_sim float32 [M]).
    """
    sims = np.einsum("rd,rcd->rc", a64, b64[cand_idx], optimize=True)
    sims32 = sims.astype(np.float32)
    t = np.clip(np.float32(1.0) - sims32, np.float32(1e-6), None).astype(np.float32)
    dist = (SQRT_2 * np.sqrt(t)).astype(np.float32)
    dmin = dist.min(axis=1, keepdims=True)
    tie = dist == dmin
    gi = np.where(tie, cand_idx, np.int64(1) << 40)
    win_idx = gi.min(axis=1)
    wpos = np.argmax(gi == win_idx[:, None], axis=1)
    win_sim = sims32[np.arange(len(sims32)), wpos]
    return win_idx, win_sim


def _match_batch_host(idx_core, d0b, d1b):
    """Reproduce reference _match_batch from one core's candidate indices.

    idx_core: [PART, 2*NSTRIP*TOPK] u16 device output for this batch.
    """
    # [p, phase, m, k] -> row r = m*PART + p
    I = idx_core.reshape(PART, 2, NSTRIP, TOPK).transpose(1, 2, 0, 3)
    I = I.reshape(2, M, TOPK).astype(np.int64)

    d0_64 = d0b.astype(np.float64)
    d1_64 = d1b.astype(np.float64)
    n_amin, sim_row = _pick_winners(I[0], d0_64, d1_64)
    m_amin, _ = _pick_winners(I[1], d1_64, d0_64)

    rng_m = np.arange(M, dtype=np.int64)
    mask = m_amin[n_amin] == rng_m

    t = np.clip(np.float32(1.0) - sim_row, np.float32(1e-6), None).astype(np.float32)
    dist_w = (SQRT_2 * np.sqrt(t)).astype(np.float32)
    score = (np.float32(1.0) / (np.float32(1.0) + dist_w)).astype(np.float32)

    m0 = np.where(mask, n_amin, -1).astype(np.int32)
    ms0 = np.where(mask, score, np.float32(0.0)).astype(np.float32)

    m1 = np.full(N, -1, dtype=np.int32)
    ms1 = np.zeros(N, dtype=np.float32)
    sel = np.flatnonzero(mask)
    m1[n_amin[sel]] = sel.astype(np.int32)
    ms1[n_amin[sel]] = score[sel]
    return m0, ms0, m1, ms1


def kernel(kpts0, desc0, kpts1, desc1):
    desc0 = np.asarray(desc0, dtype=np.float32)
    desc1 = np.asarray(desc1, dtype=np.float32)
    assert desc0.shape == (B, M, D) and desc1.shape == (B, N, D)

    at_g, bt_g = stage_inputs(desc0, desc1)
    idx_g = run_device(at_g, bt_g)

    m0 = np.empty((B, M), np.int32)
    ms0 = np.empty((B, M), np.float32)
    m1 = np.empty((B, N), np.int32)
    ms1 = np.empty((B, N), np.float32)
    for b in range(B):
        m0[b], ms0[b], m1[b], ms1[b] = _match_batch_host(
            idx_g[b * PART:(b + 1) * PART], desc0[b], desc1[b]
        )
    return m0, ms0, m1, ms1
